# revision 6
# baseline (speedup 1.0000x reference)
"""Trainium2 Bass kernel for CrossAttention (SD-style).

Math (per batch item b, all on one NeuronCore; data-parallel over batch):
    x    = query[b] viewed as [C, N] = [320, 4096]  (NCHW is token-transposed already)
    kvT  = key_value[b].T                [1024, 77]
    kT   = Wk.T @ kvT                    [512, 77]
    v    = key_value[b] @ Wv             [77, 512]
    M_h  = Wq_h @ kT_h                   [320, 77]   (q-projection folded into keys)
    per head h (64 dims):
        logitsT_h = M_h.T @ x            [77, 4096]  == (k_h q_h^T) un-scaled
        expT_h    = exp(logitsT_h / 8)
        out'_h    = v_h.T @ expT_h       [64, 4096]  (unnormalized)
        sums_h    = ones.T @ expT_h      (replicated to 64 rows)
        outT_h    = out'_h * (1/sums_h)  (DVE reciprocal + multiply)
    outT = Wo.T @ outT + bo              [320, 4096] == output[b] in NCHW

The hot-loop matmuls run in float32r (single-pass PE: 1 cycle/row vs 4 for
float32 at free-dim >= 512). fp32r ISA restrictions handled here:
  - moving-operand innermost count must be even -> kT padded to 78 (pad = 0)
  - dst start_partition must be 0 -> head pairs are stacked vertically in one
    PSUM tile by accumulating two M=128 matmuls whose stationary operands are
    zero-padded to the complementary 64 columns.
Small one-time prep matmuls (kvT/kT/v/WqT) stay in exact fp32.
"""

import functools
import os
import sys

for _p in ("/opt/trn_rl_repo",):
    if os.path.isdir(_p) and _p not in sys.path:
        sys.path.insert(0, _p)

import numpy as np

import concourse.bass as bass
import concourse.mybir as mybir
from concourse import bacc
import concourse.tile as tile
from concourse.bass_utils import run_bass_kernel_spmd
from concourse.masks import make_identity

B, C, HW2 = 8, 320, 4096
SKV, DKV = 77, 1024
SKP = 78  # SKV padded even for fp32r moving-operand rule
HEADS, DH, INNER = 8, 64, 512
NT = 512
N_TILES = HW2 // NT
SCALE = DH**-0.5
F32 = mybir.dt.float32
MDT = mybir.dt.float32r


@functools.lru_cache(maxsize=1)
def _build():
    nc = bacc.Bacc("TRN2", target_bir_lowering=False, debug=False)
    xT = nc.dram_tensor("xT", [C, HW2], MDT, kind="ExternalInput")
    kv = nc.dram_tensor("kv", [SKV, DKV], F32, kind="ExternalInput")
    Wq = nc.dram_tensor("Wq", [C, INNER], F32, kind="ExternalInput")
    Wk = nc.dram_tensor("Wk", [DKV, INNER], F32, kind="ExternalInput")
    Wv = nc.dram_tensor("Wv", [DKV, INNER], F32, kind="ExternalInput")
    Wo = nc.dram_tensor("Wo", [INNER, C], MDT, kind="ExternalInput")
    bo = nc.dram_tensor("bo", [C], F32, kind="ExternalInput")
    outT = nc.dram_tensor("outT", [C, HW2], F32, kind="ExternalOutput")

    Exp = mybir.ActivationFunctionType.Exp
    Ident = mybir.ActivationFunctionType.Identity

    with tile.TileContext(nc) as tc:
        with (
            tc.tile_pool(name="consts", bufs=1) as consts,
            tc.tile_pool(name="xp", bufs=2) as xp,
            tc.tile_pool(name="ep", bufs=3) as ep,
            tc.tile_pool(name="op", bufs=2) as op_,
            tc.tile_pool(name="fp", bufs=2) as fp,
            tc.tile_pool(name="ps_mm", bufs=2, space="PSUM") as ps_mm,
            tc.tile_pool(name="ps_l", bufs=2, space="PSUM") as ps_l,
            tc.tile_pool(name="ps_vs", bufs=1, space="PSUM") as ps_vs,
        ):
            # ---- constants / weights ----
            wq = consts.tile([128, 3, INNER], F32)
            nc.sync.dma_start(wq[:, 0, :], Wq[0:128, :])
            nc.sync.dma_start(wq[:, 1, :], Wq[128:256, :])
            nc.sync.dma_start(wq[0:64, 2, :], Wq[256:320, :])
            wk = consts.tile([128, 8, INNER], F32)
            nc.sync.dma_start(wk[:], Wk.rearrange("(ko ki) n -> ki ko n", ki=128))
            wv = consts.tile([128, 8, INNER], F32)
            nc.sync.dma_start(wv[:], Wv.rearrange("(ko ki) n -> ki ko n", ki=128))
            wo = consts.tile([128, 4, C], MDT)
            nc.sync.dma_start(wo[:], Wo.rearrange("(ko ki) n -> ki ko n", ki=128))
            bo_sb = consts.tile([128, 3], F32)
            nc.sync.dma_start(bo_sb[:, 0:1], bo[0:128, None])
            nc.sync.dma_start(bo_sb[:, 1:2], bo[128:256, None])
            nc.sync.dma_start(bo_sb[0:64, 2:3], bo[256:320, None])
            kv_sb = consts.tile([SKV, DKV], F32)
            nc.sync.dma_start(kv_sb[:], kv[:, :])
            ident = consts.tile([128, 128], F32)
            make_identity(nc, ident)
            zf = consts.tile([128, 8], F32)
            nc.vector.memset(zf, 0.0)

            # ---- prep (exact fp32): kvT, kT, v, WqT, then fp32r M ----
            # kvT[:, t, :] = key_value[:, 128t:128(t+1)].T  via PE transpose
            kvT = consts.tile([128, 8, SKV], F32)
            for t in range(8):
                tp = ps_mm.tile([128, SKV], F32, tag="mm")
                nc.tensor.transpose(
                    tp, kv_sb[:, 128 * t : 128 * (t + 1)], ident[0:SKV, 0:SKV]
                )
                nc.vector.tensor_copy(kvT[:, t, :], tp)
            # kT = Wk.T @ kvT : [512, 77] as [128, 4(pair), 78] (pad col 77 = 0)
            kT = consts.tile([128, 4, SKP], MDT)
            for m in range(4):
                ps = ps_mm.tile([128, SKV], F32, tag="mm")
                for k in range(8):
                    nc.tensor.matmul(
                        ps,
                        wk[:, k, 128 * m : 128 * (m + 1)],
                        kvT[:, k, :],
                        start=(k == 0),
                        stop=(k == 7),
                    )
                nc.vector.tensor_copy(kT[:, m, 0:SKV], ps)
                nc.vector.tensor_copy(kT[:, m, SKV:SKP], zf[:, 0:1])
            # v = key_value @ Wv : [77, 512]
            vps = ps_mm.tile([SKV, INNER], F32, tag="mm")
            for k in range(8):
                nc.tensor.matmul(
                    vps, kvT[:, k, :], wv[:, k, :], start=(k == 0), stop=(k == 7)
                )
            # Stationaries for the out'/sums matmuls, zero-padded to M=128:
            #   stage[:, h, 64*(h%2):+64] = v_h ; stage[:, 8, 0:64] = 1 (even sums)
            #   stage[:, 9, 64:128] = 1 (odd sums)
            stage = consts.tile([SKV, 10, 128], F32)
            nc.vector.memset(stage, 0.0)
            nc.vector.memset(stage[:, 8, 0:64], 1.0)
            nc.vector.memset(stage[:, 9, 64:128], 1.0)
            for h in range(HEADS):
                off = 64 * (h % 2)
                nc.vector.tensor_copy(
                    stage[:, h, off : off + 64], vps[:, 64 * h : 64 * h + 64]
                )
            v2 = consts.tile([SKV, 10, 128], MDT)
            nc.vector.tensor_copy(v2, stage)
            # WqT [512, 320] as [128, 4(mo), 320] via PE transposes of Wq
            wqT = consts.tile([128, 4, C], MDT)
            for ct in range(3):
                CP = 128 if ct < 2 else 64
                for mo in range(4):
                    tp = ps_mm.tile([128, 128], F32, tag="mm")
                    nc.tensor.transpose(
                        tp[:, 0:CP],
                        wq[0:CP, ct, 128 * mo : 128 * (mo + 1)],
                        ident[0:CP, 0:CP],
                    )
                    nc.vector.tensor_copy(
                        wqT[:, mo, 128 * ct : 128 * ct + CP], tp[:, 0:CP]
                    )
            # M_h = Wq_h @ kT_h : [320, 78] per head (col 77 = 0), fp32r
            m_sb = consts.tile([128, 3, HEADS, SKP], MDT)
            for h in range(HEADS):
                po = slice(64 * (h % 2), 64 * (h % 2) + 64)
                for ko in range(3):
                    KP = 128 if ko < 2 else 64
                    ps = ps_mm.tile([128, SKP], F32, tag="mm")
                    nc.tensor.matmul(
                        ps[0:KP, :],
                        wqT[po, h // 2, 128 * ko : 128 * ko + KP],
                        kT[po, h // 2, :],
                        start=True,
                        stop=True,
                    )
                    nc.vector.tensor_copy(m_sb[0:KP, ko, h, :], ps[0:KP, :])

            # ---- main loop over token tiles ----
            for n in range(N_TILES):
                nsl = slice(NT * n, NT * (n + 1))
                xt = xp.tile([128, 3, NT], MDT)
                nc.sync.dma_start(xt[:, 0, :], xT[0:128, nsl])
                nc.sync.dma_start(xt[:, 1, :], xT[128:256, nsl])
                nc.sync.dma_start(xt[0:64, 2, :], xT[256:320, nsl])

                # attention per head pair (heads 2j / 2j+1 stacked in psum partitions)
                o_sb = op_.tile([128, 4, NT], MDT)
                for j in range(4):
                    h0, h1 = 2 * j, 2 * j + 1
                    lps = ps_l.tile([SKP, 2, NT], F32)
                    for hh in range(2):
                        for ko in range(3):
                            KP = 128 if ko < 2 else 64
                            nc.tensor.matmul(
                                lps[:, hh, :],
                                m_sb[0:KP, ko, 2 * j + hh, :],
                                xt[0:KP, ko, :],
                                start=(ko == 0),
                                stop=(ko == 2),
                            )
                    et = ep.tile([SKP, 2, NT], MDT)
                    nc.scalar.activation(et, lps[:, :, :], Exp, scale=SCALE)
                    vs = ps_vs.tile([128, 2, NT], F32)
                    nc.tensor.matmul(
                        vs[:, 0, :], v2[:, h0, :], et[0:SKV, 0, :],
                        start=True, stop=False,
                    )
                    nc.tensor.matmul(
                        vs[:, 0, :], v2[:, h1, :], et[0:SKV, 1, :],
                        start=False, stop=True,
                    )
                    nc.tensor.matmul(
                        vs[:, 1, :], v2[:, 8, :], et[0:SKV, 0, :],
                        start=True, stop=False,
                    )
                    nc.tensor.matmul(
                        vs[:, 1, :], v2[:, 9, :], et[0:SKV, 1, :],
                        start=False, stop=True,
                    )
                    rt = ep.tile([128, NT], F32, tag="rt")
                    nc.vector.reciprocal(rt, vs[:, 1, :])
                    nc.vector.tensor_tensor(
                        o_sb[:, j, :], vs[:, 0, :], rt, mybir.AluOpType.mult
                    )

                # output projection + bias
                ft = fp.tile([128, 3, NT], F32)
                for cti in range(3):
                    CP = 128 if cti < 2 else 64
                    csl = slice(128 * cti, 128 * cti + CP)
                    wps = ps_mm.tile([128, NT], F32, tag="mm")
                    for k in range(4):
                        nc.tensor.matmul(
                            wps[0:CP, :],
                            wo[:, k, csl],
                            o_sb[:, k, :],
                            start=(k == 0),
                            stop=(k == 3),
                        )
                    nc.scalar.activation(
                        ft[0:CP, cti, :],
                        wps[0:CP, :],
                        Ident,
                        bias=bo_sb[0:CP, cti : cti + 1],
                        scale=1.0,
                    )
                nc.sync.dma_start(outT[0:128, nsl], ft[:, 0, :])
                nc.sync.dma_start(outT[128:256, nsl], ft[:, 1, :])
                nc.sync.dma_start(outT[256:320, nsl], ft[0:64, 2, :])
    nc.compile()
    return nc


def _in_maps(query, key_value, Wq, Wk, Wv, Wo, bo):
    query = np.ascontiguousarray(np.asarray(query, np.float32))
    key_value = np.ascontiguousarray(np.asarray(key_value, np.float32))
    shared = {
        "Wq": np.ascontiguousarray(np.asarray(Wq, np.float32)),
        "Wk": np.ascontiguousarray(np.asarray(Wk, np.float32)),
        "Wv": np.ascontiguousarray(np.asarray(Wv, np.float32)),
        "Wo": np.ascontiguousarray(np.asarray(Wo, np.float32)),
        "bo": np.ascontiguousarray(np.asarray(bo, np.float32)),
    }
    maps = []
    for b in range(B):
        m = dict(shared)
        m["xT"] = np.ascontiguousarray(query[b].reshape(C, HW2))
        m["kv"] = np.ascontiguousarray(key_value[b])
        maps.append(m)
    return maps


def kernel(query, key_value, Wq, Wk, Wv, Wo, bo, **kwargs):
    nc = _build()
    maps = _in_maps(query, key_value, Wq, Wk, Wv, Wo, bo)
    res = run_bass_kernel_spmd(nc, maps, core_ids=list(range(B)), **kwargs)
    out = np.stack(
        [res.results[b]["outT"].reshape(C, 64, 64) for b in range(B)]
    ).astype(np.float32)
    return out
